# revision 1
# baseline (speedup 1.0000x reference)
"""Trainium2 Bass kernel for AdvancedMoEMixtureLoRA.

Reference computation (per token t of N = 4*2048 = 8192, D = 4096):
    z        = x @ A_w.T                       [N, 16]
    M        = 8 * (x @ M_w.T + M_b)           [N, 256] -> [N, 16, 16]
    z_mixed  = M @ z  (per token matvec)       [N, 16]
    out      = 128 * z_mixed @ B_w.T           [N, 4096]

Strategy: pure data parallel over tokens (1024 tokens per core, weights
replicated, no collectives).  Host-side prep (free, not on HW critical
path): transpose x to d-major, cast everything to bf16, fuse A_w/M_w
into one [4096, 272] weight, fold all scalar factors into the weights.

Per-core kernel, per 128-token chunk:
  - 32 accumulating matmuls (stationary = xT d-tile, moving = fused W)
    + one K=1 matmul adding the M_b bias row -> PSUM [128, 272]
    (cols 0:256 = M, cols 256:272 = z)
  - DVE mixing: P = M * broadcast(z), grouped reduce over j -> z_mixed
  - PE transpose z_mixed -> [16, 128], then 8 matmuls against
    B_w.T [16, 4096] -> out rows [128, 4096] in PSUM
  - DVE/ACT evacuate PSUM -> SBUF bf16, DMA store contiguous rows
"""

import sys

if "/opt/trn_rl_repo" not in sys.path:
    sys.path.insert(0, "/opt/trn_rl_repo")

import ml_dtypes
import numpy as np

import concourse.bass as bass
import concourse.tile as tile
from concourse import bacc, mybir
from concourse.bass_utils import run_bass_kernel_spmd

N_CORES = 8
B, S, D = 4, 2048, 4096
N_TOK = B * S                # 8192
TPC = N_TOK // N_CORES       # tokens per core = 1024
CHUNK = 128                  # tokens per PSUM chunk
NCHUNK = TPC // CHUNK        # 8
RH = 16                      # lora rank*heads
MDIM = RH * RH               # 256
WCOLS = MDIM + RH            # 272 fused output cols (M | z)
KD = D // 128                # 32 d-chunks
OUT_D = 4096

BF = mybir.dt.bfloat16
F32 = mybir.dt.float32
NPBF = ml_dtypes.bfloat16


def build_nc():
    nc = bacc.Bacc("TRN2", target_bir_lowering=False, debug=False)
    xT = nc.dram_tensor("xT", [D, TPC], BF, kind="ExternalInput").ap()
    wT = nc.dram_tensor("wT", [D, WCOLS], BF, kind="ExternalInput").ap()
    mbr = nc.dram_tensor("mbr", [1, WCOLS], BF, kind="ExternalInput").ap()
    bT = nc.dram_tensor("bT", [RH, OUT_D], BF, kind="ExternalInput").ap()
    ones = nc.dram_tensor("ones", [1, CHUNK], BF, kind="ExternalInput").ap()
    ident = nc.dram_tensor("ident", [CHUNK, CHUNK], F32, kind="ExternalInput").ap()
    out = nc.dram_tensor("out", [TPC, OUT_D], BF, kind="ExternalOutput").ap()

    with tile.TileContext(nc) as tc:
        with (
            tc.tile_pool(name="xpool", bufs=1) as xpool,
            tc.tile_pool(name="wpool", bufs=1) as wpool,
            tc.tile_pool(name="cpool", bufs=1) as cpool,
            tc.tile_pool(name="mix", bufs=2) as mixpool,
            tc.tile_pool(name="osb", bufs=3) as opool,
            tc.tile_pool(name="am", bufs=2, space="PSUM") as ampool,
            tc.tile_pool(name="tp", bufs=1, space="PSUM") as tpool,
            tc.tile_pool(name="bp", bufs=5, space="PSUM") as bpool,
        ):
            # weights / constants
            wsb = wpool.tile([128, KD, WCOLS], BF)
            nc.sync.dma_start(wsb[:], wT.rearrange("(k p) m -> p k m", p=128))
            mbsb = cpool.tile([1, WCOLS], BF)
            nc.sync.dma_start(mbsb[:], mbr)
            btsb = cpool.tile([RH, OUT_D], BF)
            nc.sync.dma_start(btsb[:], bT)
            onesb = cpool.tile([1, CHUNK], BF)
            nc.sync.dma_start(onesb[:], ones)
            idsb = cpool.tile([CHUNK, CHUNK], F32)
            nc.sync.dma_start(idsb[:], ident)

            # x, transposed on host: [4096, 1024] -> 32 tiles [128, 1024]
            xsb = xpool.tile([128, KD, TPC], BF)
            xv = xT.rearrange("(k p) t -> p k t", p=128)
            for g in range(8):
                nc.sync.dma_start(xsb[:, 4 * g:4 * (g + 1), :], xv[:, 4 * g:4 * (g + 1), :])

            for c in range(NCHUNK):
                tok = slice(c * CHUNK, (c + 1) * CHUNK)
                # fused A/M matmul: out [128 tok, 272], contract over d
                am = ampool.tile([128, WCOLS], F32)
                for k in range(KD):
                    nc.tensor.matmul(
                        am[:], lhsT=xsb[:, k, tok], rhs=wsb[:, k, :],
                        start=(k == 0), stop=False,
                    )
                # bias row via K=1 matmul (ones.T @ mb_row)
                nc.tensor.matmul(am[:], lhsT=onesb[:], rhs=mbsb[:], start=False, stop=True)

                # z -> SBUF (scalar engine, tiny)
                z_sb = mixpool.tile([128, RH], F32, tag="z")
                nc.scalar.copy(z_sb[:], am[:, MDIM:WCOLS])

                # P[p, i, j] = M[p, i, j] * z[p, j]
                p_sb = mixpool.tile([128, MDIM], BF, tag="p")
                nc.vector.tensor_mul(
                    p_sb[:].rearrange("p (i j) -> p i j", i=RH),
                    am[:, 0:MDIM].rearrange("p (i j) -> p i j", i=RH),
                    z_sb[:].unsqueeze(1).broadcast_to([128, RH, RH]),
                )
                # z_mixed[p, i] = sum_j P[p, i, j]
                zm = mixpool.tile([128, RH], F32, tag="zm")
                nc.vector.tensor_reduce(
                    zm[:], p_sb[:].rearrange("p (i j) -> p i j", i=RH),
                    axis=mybir.AxisListType.X, op=mybir.AluOpType.add,
                )

                # transpose z_mixed -> [16, 128] for the B matmul stationary
                zt_ps = tpool.tile([RH, CHUNK], F32)
                nc.tensor.transpose(zt_ps[:], zm[:], idsb[:])
                zt_sb = mixpool.tile([RH, CHUNK], BF, tag="zt")
                nc.scalar.copy(zt_sb[:], zt_ps[:])

                # out rows = z_mixed @ B_w.T : 8 matmuls of [16,128].T @ [16,512]
                o_sb = opool.tile([128, OUT_D], BF)
                for ob in range(8):
                    osl = slice(ob * 512, (ob + 1) * 512)
                    bp = bpool.tile([128, 512], F32)
                    nc.tensor.matmul(bp[:], lhsT=zt_sb[:], rhs=btsb[:, osl], start=True, stop=True)
                    # split PSUM evacuation between DVE and ACT
                    if ob % 2 == 0:
                        nc.vector.tensor_copy(o_sb[:, osl], bp[:])
                    else:
                        nc.scalar.copy(o_sb[:, osl], bp[:])
                nc.sync.dma_start(out[tok, :], o_sb[:])

    nc.compile()
    return nc


_NC = None


def _get_nc():
    global _NC
    if _NC is None:
        _NC = build_nc()
    return _NC


def make_in_maps(x, A_w, B_w, M_w, M_b):
    x = np.asarray(x, dtype=np.float32)
    A_w = np.asarray(A_w, dtype=np.float32)
    B_w = np.asarray(B_w, dtype=np.float32)
    M_w = np.asarray(M_w, dtype=np.float32)
    M_b = np.asarray(M_b, dtype=np.float32)

    # fold scales: M' = x @ (8 M_w).T + 8 M_b ; out = z_mixed @ (128 B_w).T
    W = np.concatenate([8.0 * M_w, A_w], axis=0)              # [272, 4096]
    wT_np = np.ascontiguousarray(W.T).astype(NPBF)            # [4096, 272]
    mb_np = np.concatenate([8.0 * M_b, np.zeros(RH, np.float32)]).reshape(1, WCOLS).astype(NPBF)
    bT_np = np.ascontiguousarray((128.0 * B_w).T).astype(NPBF)  # [16, 4096]
    ones_np = np.ones((1, CHUNK), dtype=NPBF)
    id_np = np.eye(CHUNK, dtype=np.float32)

    xf = x.reshape(N_TOK, D)
    in_maps = []
    for c in range(N_CORES):
        shard = xf[c * TPC:(c + 1) * TPC]                     # [1024, 4096]
        xT_np = np.ascontiguousarray(shard.T).astype(NPBF)    # [4096, 1024]
        in_maps.append({
            "xT": xT_np, "wT": wT_np, "mbr": mb_np, "bT": bT_np,
            "ones": ones_np, "ident": id_np,
        })
    return in_maps


def assemble_out(results):
    outs = [np.asarray(results[i]["out"], dtype=np.float32) for i in range(N_CORES)]
    return np.concatenate(outs, axis=0).reshape(B, S, OUT_D)


def kernel(x, A_w, B_w, M_w, M_b):
    nc = _get_nc()
    in_maps = make_in_maps(x, A_w, B_w, M_w, M_b)
    res = run_bass_kernel_spmd(nc, in_maps, core_ids=list(range(N_CORES)))
    return assemble_out(res.results)
